# revision 25
# baseline (speedup 1.0000x reference)
"""Trainium2 Bass kernel for nn_AlphaModel (gnn_message_passing).

Math (per edge n, P=3):
    M       = rel_mu[rels[n]] + rel_sigma[rels[n]] * eps_M[n]        [3,3]
    cp      = softmax(M @ child[n])                                  [3]
    masks   from row sums of child / prnt
    s       = 42 * max(.01, cos(prnt, cp)) / H(normalize(max(.01, prnt+cp)))
    alpha   = ((1-beta) * prnt + beta * cp) * s          (alpha_mask rows)
    c2c     = cp                                         (copy_mask rows)

Sharding: pure data parallel over the edge dim across 8 NeuronCores.
The rel_mu/rel_sigma tables in this problem are degenerate (all 20 rows
identical), so M = MU + sigma*eps needs no per-edge gather; detected at
runtime, MU baked as immediates. General tables fall back to a host-side
gather of per-edge mu/sigma rows (extra planar DMA inputs).

Layout: PLANAR (feature-major). The host transposes/packs inputs into
  pcb  [7, NS]  = child(3) | prnt(3) | beta(1)
  eps  [9, NS]
and the device writes one packed output
  out8 [8, NS]  = c2c(3) | alpha(3) | copy_mask(1) | alpha_mask(1)
so each tile is 2 loads + 1 store and every operand is a contiguous
[128, T] plane (or a small strided stack of planes).

Device-side identities (validated vs reference to ~1e-5):
  - cos(p, cp) == cos(p, e) for e = exp(logits)  (scale invariance)
  - H(z/zs) = ln(zs) - (1/zs) * sum(z ln z), with z = relu(p+cp-.01)+.01
    and the +0.03 of zs folded into activation biases
  - 1/x (and rsqrt, and 42/x) as exp(-a*ln(x)+b) on the scalar engine
    (the custom-DVE fast reciprocal does not compile in this container)
  - norm guard: cos_raw = dot * rsqrt(sp*sc + 1e-30); dot==0 exactly when
    a norm is 0, matching the reference's where().

A post-pass (split_waits) hoists multi-semaphore waits onto dedicated
event-sem instructions: this container's walrus rejects instructions
carrying >1 sync wait.
"""

import os
import sys

sys.path.insert(0, "/opt/trn_rl_repo")

import numpy as np

import bass_rust
import concourse.bass as bass
import concourse.mybir as mybir
import concourse.tile as tile
from concourse.bass_utils import run_bass_kernel_spmd

PARTS = 128
T_COL = 784          # edges per partition per tile (even: bf16 2x align)
N_TILES = 5          # tiles per core
N_CORES = 8

f32 = mybir.dt.float32
bf16 = mybir.dt.bfloat16
Alu = mybir.AluOpType
Act = mybir.ActivationFunctionType

LAST_RESULT = None  # BassKernelResults of the most recent run (for test.py)

# setup_inputs() defaults, used if the harness omits the tiny tables.
_MU_DEFAULT = np.eye(3, dtype=np.float32)
_MU_DEFAULT[1, :] = [-0.25, 0.5, -0.25]

# ---------------------------------------------------------------------------
# split_waits post-pass
# ---------------------------------------------------------------------------
_WSPLIT_N = [0]


def _wait_carrier(engine, wait):
    _WSPLIT_N[0] += 1
    ev = mybir.InstEventSemaphore(name=f"WSPLIT-{_WSPLIT_N[0]}", ins=[], outs=[])
    ev.engine = engine
    ev.sync_info = bass_rust.SyncInfo(on_wait=[wait], on_update=[])
    return ev


def split_waits(nc, keep_on_control=1):
    for fn in nc.m.functions:
        for blk in fn.blocks:
            out = []
            for ins in blk.instructions:
                si = ins.sync_info
                waits = list(si.on_wait) if (si and si.on_wait) else []
                is_ctrl = type(ins).__name__ in (
                    "InstEventSemaphore",
                    "InstDrain",
                    "InstUnconditionalBranch",
                    "InstCompareAndBranch",
                    "InstBranchHint",
                )
                keep = keep_on_control if is_ctrl else 0
                if len(waits) > keep:
                    cut = len(waits) - keep
                    for w in waits[:cut]:
                        out.append(_wait_carrier(ins.engine, w))
                    ins.sync_info = bass_rust.SyncInfo(
                        on_wait=waits[cut:], on_update=list(si.on_update or [])
                    )
                out.append(ins)
            blk.instructions = out
    return nc



def build_graph(mu9, sg9, general, t_col=T_COL, n_tiles=N_TILES):
    """Per-core graph, planar layout.

    Fast path (degenerate tables): bf16 on every numerically-safe chain
    (logits/softmax/cosine/blend; DVE gets 2x-4x modes), f32 where rounding
    would amplify (ln values, entropy sums, z-sums). eps is cast f32->bf16
    by the SWDGE DMA in flight. GpSimd does no elementwise compute: it
    shares SBUF ports with the DVE and the two serialize, not overlap.
    """
    T = t_col
    NS = PARTS * T * n_tiles
    sigma_is_one = (not general) and bool(np.all(sg9 == 1.0))

    nc = bass.Bass()
    in_dt = f32 if general else bf16
    pcb_h = nc.declare_dram_parameter("pcb", [7, NS], in_dt, isOutput=False)
    eps_h = nc.declare_dram_parameter("eps", [9, NS], in_dt, isOutput=False)
    if general:
        murow_h = nc.declare_dram_parameter("murow", [9, NS], f32, isOutput=False)
        sgrow_h = nc.declare_dram_parameter("sgrow", [9, NS], f32, isOutput=False)
    outv_h = nc.declare_dram_parameter("outv", [6, NS], bf16, isOutput=True)
    outm_h = nc.declare_dram_parameter("outm", [2, NS], bf16, isOutput=True)

    def dram_tile(handle, base):
        return (
            handle[:, base : base + PARTS * T]
            .rearrange("k (p t) -> k p t", p=PARTS)
            .transpose([1, 0, 2])
        )

    with tile.TileContext(nc) as tc:
        with (
            tc.tile_pool(name="const", bufs=1) as cpool,
            tc.tile_pool(name="io", bufs=2) as io,
            tc.tile_pool(name="work", bufs=1) as wk,
            tc.tile_pool(name="work2", bufs=2) as wk2,
        ):
            bias_m01 = cpool.tile([PARTS, 1], f32)
            nc.gpsimd.memset(bias_m01[:], -0.01)
            bias_p01 = cpool.tile([PARTS, 1], f32)
            nc.gpsimd.memset(bias_p01[:], 0.01)
            bias_p03 = cpool.tile([PARTS, 1], f32)
            nc.gpsimd.memset(bias_p03[:], 0.03)
            bias_tiny = cpool.tile([PARTS, 1], f32)
            nc.gpsimd.memset(bias_tiny[:], 1e-30)
            bias_ln42 = cpool.tile([PARTS, 1], f32)
            nc.gpsimd.memset(bias_ln42[:], float(np.log(42.0)))

            total_cols = T * n_tiles
            if n_tiles >= 2:
                t_plan = [T // 2, T // 2] + [T] * (n_tiles - 1)
            else:
                t_plan = [T] * n_tiles
            assert sum(t_plan) == total_cols and all(t % 2 == 0 for t in t_plan)
            base = 0
            T_MAX = T
            for it, t_cur in enumerate(t_plan):
                T = t_cur

                pdt = bf16 if not general else f32
                pcb_t = io.tile([PARTS, 7 * T], pdt, tag="pcb_t")
                nc.sync.dma_start(pcb_t[:], dram_tile(pcb_h, base))
                edt = bf16 if not general else f32
                e_t = io.tile([PARTS, 9 * T], edt, tag="e_t")
                nc.sync.dma_start(e_t[:], dram_tile(eps_h, base))
                if general:
                    mr_t = io.tile([PARTS, 9 * T], f32, tag="mr_t")
                    sr_t = io.tile([PARTS, 9 * T], f32, tag="sr_t")
                    nc.sync.dma_start(mr_t[:], dram_tile(murow_h, base))
                    nc.sync.dma_start(sr_t[:], dram_tile(sgrow_h, base))
                outv = io.tile([PARTS, 6 * T], bf16, tag="outv")
                outm = io.tile([PARTS, 2 * T], bf16, tag="outm")

                def sl(tl, k0, k1):
                    return tl[:, k0 * T : k1 * T]

                c3 = sl(pcb_t, 0, 3)
                p3 = sl(pcb_t, 3, 6)
                bt = sl(pcb_t, 6, 7)

                # fast path: pcb_t is already bf16; alias planes
                cpb6 = pcb_t[:, : 6 * T]
                cb3 = sl(pcb_t, 0, 3)
                pb3 = sl(pcb_t, 3, 6)
                bbf = sl(pcb_t, 6, 7)

                # ---- logits_i = sum_j (mu_ij + sg_ij*eps_ij)*c_j  [bf16] --
                epl = [sl(e_t, k, k + 1) for k in range(9)]
                if general:
                    nc.vector.tensor_tensor(e_t[:], e_t[:], sr_t[:], Alu.mult)
                    nc.vector.tensor_tensor(e_t[:], e_t[:], mr_t[:], Alu.add)
                elif not sigma_is_one:
                    for k in range(9):
                        nc.vector.tensor_scalar(
                            epl[k], epl[k], float(sg9[k]), float(mu9[k]),
                            Alu.mult, Alu.add,
                        )
                else:
                    for k in range(9):
                        # bf16 tensor_scalar: 4x mode
                        nc.vector.tensor_scalar(
                            epl[k], epl[k], float(mu9[k]), None, Alu.add
                        )
                e33 = e_t[:].rearrange("p (a b t) -> p a b t", a=3, b=3)
                for k in range(9):
                    nc.vector.tensor_tensor(
                        epl[k], epl[k], sl(pcb_t, k % 3, k % 3 + 1), Alu.mult
                    )
                # ---- exp / dots: exd = ex(3) | dpm(3)  [bf16] ------------
                exd = wk2.tile([PARTS, 6 * T], bf16, tag="exd")
                ex3t = sl(exd, 0, 3)
                lg3 = ex3t.rearrange("p (i t) -> p i t", i=3)
                nc.vector.tensor_tensor(
                    lg3, e33[:, :, 0, :], e33[:, :, 1, :], Alu.add
                )
                nc.vector.tensor_tensor(lg3, lg3, e33[:, :, 2, :], Alu.add)
                nc.scalar.activation(ex3t, ex3t, Act.Exp)
                nc.vector.tensor_tensor(sl(exd, 3, 6), pb3, ex3t, Alu.mult)
                sdp = wk.tile([PARTS, 2 * T], bf16, tag="sdp")
                sdp2 = sdp[:].rearrange("p (g t) -> p g t", g=2)
                exd23 = exd[:].rearrange("p (g i t) -> p g i t", g=2, i=3)
                nc.vector.tensor_tensor(
                    sdp2, exd23[:, :, 0, :], exd23[:, :, 1, :], Alu.add
                )
                nc.vector.tensor_tensor(sdp2, sdp2, exd23[:, :, 2, :], Alu.add)
                se = sl(sdp, 0, 1)
                dp = sl(sdp, 1, 2)

                # ---- r = 1/se (ln in f32!); cp = ex*r  [bf16] ------------
                rln = wk2.tile([PARTS, T], f32, tag="rln")
                nc.scalar.activation(rln[:], se, Act.Ln)
                r = wk2.tile([PARTS, T], bf16, tag="r")
                nc.scalar.activation(r[:], rln[:], Act.Exp, scale=-1.0)
                cp = wk.tile([PARTS, 3 * T], bf16, tag="cp")
                cp3 = cp[:].rearrange("p (i t) -> p i t", i=3)
                for i in range(3):
                    nc.vector.tensor_tensor(
                        sl(cp, i, i + 1), sl(exd, i, i + 1), r[:], Alu.mult
                    )

                # ---- masks (bf16 sums are exactly-zero iff f32 sums are) -
                cps2 = wk.tile([PARTS, 2 * T], bf16, tag="cps2")
                cps22 = cps2[:].rearrange("p (g t) -> p g t", g=2)
                cpb23 = cpb6.rearrange("p (g i t) -> p g i t", g=2, i=3)
                nc.vector.tensor_tensor(
                    cps22, cpb23[:, :, 0, :], cpb23[:, :, 1, :], Alu.add
                )
                nc.vector.tensor_tensor(
                    cps22, cps22, cpb23[:, :, 2, :], Alu.add
                )
                csum = sl(cps2, 0, 1)
                psum = sl(cps2, 1, 2)
                cm = wk.tile([PARTS, T], bf16, tag="cm")
                nc.vector.tensor_scalar(cm[:], csum, 0.0, None, Alu.not_equal)
                copym = sl(outm, 0, 1)
                amask = sl(outm, 1, 2)
                nc.vector.scalar_tensor_tensor(
                    copym, psum, 0.0, cm[:], Alu.is_equal, Alu.mult
                )
                nc.vector.tensor_tensor(amask, cm[:], copym, Alu.subtract)

                # ---- entropy: zz = z(3) | zlnz(3), all bf16 sums ---------
                zz = wk.tile([PARTS, 6 * T], bf16, tag="zz")
                zv = sl(zz, 0, 3)
                nc.vector.tensor_tensor(zv, pb3, cp[:], Alu.add)
                nc.scalar.activation(zv, zv, Act.Relu, bias=bias_m01[:])
                lnz = wk.tile([PARTS, 3 * T], f32, tag="lnz")
                nc.scalar.activation(lnz[:], zv, Act.Ln, bias=bias_p01[:])
                z01 = wk.tile([PARTS, 3 * T], f32, tag="z01")
                nc.scalar.activation(z01[:], zv, Act.Identity, bias=bias_p01[:])
                nc.vector.tensor_tensor(sl(zz, 3, 6), z01[:], lnz[:], Alu.mult)
                zst = wk.tile([PARTS, 2 * T], bf16, tag="zst")
                zst2 = zst[:].rearrange("p (g t) -> p g t", g=2)
                zz23 = zz[:].rearrange("p (g i t) -> p g i t", g=2, i=3)
                nc.vector.tensor_tensor(
                    zst2, zz23[:, :, 0, :], zz23[:, :, 1, :], Alu.add
                )
                nc.vector.tensor_tensor(zst2, zst2, zz23[:, :, 2, :], Alu.add)
                lnzs = wk.tile([PARTS, T], f32, tag="lnzs")
                nc.scalar.activation(lnzs[:], sl(zst, 0, 1), Act.Ln, bias=bias_p03[:])
                zr = wk.tile([PARTS, T], f32, tag="zr")
                nc.scalar.activation(zr[:], lnzs[:], Act.Exp, scale=-1.0)
                ent = wk.tile([PARTS, T], f32, tag="ent")
                nc.vector.tensor_tensor(ent[:], zr[:], sl(zst, 1, 2), Alu.mult)
                nc.vector.tensor_tensor(ent[:], lnzs[:], ent[:], Alu.subtract)
                eln = wk.tile([PARTS, T], f32, tag="eln")
                nc.scalar.activation(eln[:], ent[:], Act.Ln)
                esr = wk.tile([PARTS, T], bf16, tag="esr")  # 42/ent
                nc.scalar.activation(
                    esr[:], eln[:], Act.Exp, scale=-1.0, bias=bias_ln42[:]
                )

                # ---- cosine: sq = p^2(3)|ex^2(3)  [bf16] -----------------
                sq = wk.tile([PARTS, 6 * T], bf16, tag="sq")
                nc.scalar.activation(sl(sq, 0, 3), p3, Act.Square)
                nc.scalar.activation(sl(sq, 3, 6), ex3t, Act.Square)
                ssc = wk.tile([PARTS, 2 * T], bf16, tag="ssc")
                ssc2 = ssc[:].rearrange("p (g t) -> p g t", g=2)
                sq23 = sq[:].rearrange("p (g i t) -> p g i t", g=2, i=3)
                nc.vector.tensor_tensor(
                    ssc2, sq23[:, :, 0, :], sq23[:, :, 1, :], Alu.add
                )
                nc.vector.tensor_tensor(ssc2, ssc2, sq23[:, :, 2, :], Alu.add)
                mn = wk.tile([PARTS, T], bf16, tag="mn")
                nc.vector.tensor_tensor(
                    mn[:], sl(ssc, 0, 1), sl(ssc, 1, 2), Alu.mult
                )
                mnl = wk.tile([PARTS, T], f32, tag="mnl")
                nc.scalar.activation(mnl[:], mn[:], Act.Ln, bias=bias_tiny[:])
                drr = wk.tile([PARTS, T], bf16, tag="drr")
                nc.scalar.activation(drr[:], mnl[:], Act.Exp, scale=-0.5)

                # ---- sm = (max(.01, dp*drr) * (42/ent)) * amask [bf16] ---
                sm = wk.tile([PARTS, T], bf16, tag="sm")
                nc.vector.tensor_tensor(sm[:], dp, drr[:], Alu.mult)
                nc.vector.scalar_tensor_tensor(
                    sm[:], sm[:], 0.01, esr[:], Alu.max, Alu.mult
                )
                nc.vector.tensor_tensor(sm[:], sm[:], amask, Alu.mult)

                # ---- alpha = (p + beta*(cp-p))*sm ; c2c = cp*copym -------
                av = sl(outv, 3, 6)
                nc.vector.tensor_tensor(av, cp[:], pb3, Alu.subtract)
                for i in range(3):
                    avi = sl(outv, 3 + i, 4 + i)
                    nc.vector.tensor_tensor(avi, avi, bbf, Alu.mult)
                nc.vector.tensor_tensor(av, av, pb3, Alu.add)
                for i in range(3):
                    avi = sl(outv, 3 + i, 4 + i)
                    nc.vector.tensor_tensor(avi, avi, sm[:], Alu.mult)
                for i in range(3):
                    nc.vector.tensor_tensor(
                        sl(outv, i, i + 1), sl(cp, i, i + 1), copym, Alu.mult
                    )

                nc.sync.dma_start(dram_tile(outv_h, base), outv[:])
                nc.sync.dma_start(dram_tile(outm_h, base), outm[:])
                base += PARTS * T

    split_waits(nc)
    return nc, NS


def _pack_planar(arrs, ns, dtype=np.float32):
    """[N, F_a] arrays -> list of 8 contiguous [sum F, ns] shards (padded)."""
    n = arrs[0].shape[0]
    ftot = sum(a.shape[1] for a in arrs)
    shards = []
    for i in range(N_CORES):
        lo, hi = i * ns, (i + 1) * ns
        out = np.zeros((ftot, ns), dtype=dtype)
        m = max(0, min(hi, n) - lo)
        if m > 0:
            k = 0
            for a in arrs:
                f = a.shape[1]
                out[k : k + f, :m] = a[lo : lo + m].T
                k += f
        shards.append(out)
    return shards


def kernel(
    prnt_probs,
    child_probs,
    eps_M,
    beta,
    rels=None,
    rel_mu=None,
    rel_sigma=None,
    **_unused,
):
    global LAST_RESULT
    prnt = np.asarray(prnt_probs, dtype=np.float32)
    child = np.asarray(child_probs, dtype=np.float32)
    n = prnt.shape[0]
    eps = np.asarray(eps_M, dtype=np.float32).reshape(n, 9)
    bet = np.asarray(beta, dtype=np.float32).reshape(n, 1)

    if rel_mu is None:
        rel_mu = np.tile(_MU_DEFAULT, (20, 1)).reshape(20, 3, 3)
    if rel_sigma is None:
        rel_sigma = np.ones((20, 3, 3), dtype=np.float32)
    rel_mu = np.asarray(rel_mu, dtype=np.float32)
    rel_sigma = np.asarray(rel_sigma, dtype=np.float32)

    degenerate = bool(
        np.all(rel_mu == rel_mu[0:1]) and np.all(rel_sigma == rel_sigma[0:1])
    )

    t_col = T_COL if degenerate else 360  # general path: f32 tiles, smaller T
    n_tiles = max(1, -(-n // (N_CORES * PARTS * t_col)))  # ceil
    nc, ns = build_graph(
        rel_mu[0].reshape(9),
        rel_sigma[0].reshape(9),
        general=not degenerate,
        t_col=t_col,
        n_tiles=n_tiles,
    )

    import ml_dtypes

    in_np = np.float32 if not degenerate else ml_dtypes.bfloat16
    pcb_sh = _pack_planar([child, prnt, bet], ns, dtype=in_np)
    eps_sh = _pack_planar([eps], ns, dtype=in_np)
    if not degenerate:
        ridx = np.asarray(rels).astype(np.int64)
        mr_sh = _pack_planar([rel_mu.reshape(20, 9)[ridx]], ns)
        sr_sh = _pack_planar([rel_sigma.reshape(20, 9)[ridx]], ns)
    in_maps = []
    for i in range(N_CORES):
        m = {"pcb": pcb_sh[i], "eps": eps_sh[i]}
        if not degenerate:
            m["murow"] = mr_sh[i]
            m["sgrow"] = sr_sh[i]
        in_maps.append(m)

    trace = bool(os.environ.get("ALPHA_KERNEL_TRACE"))
    res = run_bass_kernel_spmd(
        nc, in_maps, core_ids=list(range(N_CORES)), trace=trace
    )
    LAST_RESULT = res
    outs = res.results

    ov = np.concatenate([outs[i]["outv"] for i in range(N_CORES)], axis=1)
    om = np.concatenate([outs[i]["outm"] for i in range(N_CORES)], axis=1)
    c2c = np.ascontiguousarray(ov[0:3, :n].T.astype(np.float32))
    alpha = np.ascontiguousarray(ov[3:6, :n].T.astype(np.float32))
    copy_mask = om[0, :n] != 0
    alpha_mask = om[1, :n] != 0
    return copy_mask, c2c, alpha_mask, alpha


# revision 26
# speedup vs baseline: 1.2172x; 1.2172x over previous
"""Trainium2 Bass kernel for nn_AlphaModel (gnn_message_passing).

Math (per edge n, P=3):
    M       = rel_mu[rels[n]] + rel_sigma[rels[n]] * eps_M[n]        [3,3]
    cp      = softmax(M @ child[n])                                  [3]
    masks   from row sums of child / prnt
    s       = 42 * max(.01, cos(prnt, cp)) / H(normalize(max(.01, prnt+cp)))
    alpha   = ((1-beta) * prnt + beta * cp) * s          (alpha_mask rows)
    c2c     = cp                                         (copy_mask rows)

Sharding: pure data parallel over the edge dim across 8 NeuronCores.
The rel_mu/rel_sigma tables in this problem are degenerate (all 20 rows
identical), so M = MU + sigma*eps needs no per-edge gather; detected at
runtime, MU baked as immediates. General tables fall back to a host-side
gather of per-edge mu/sigma rows (extra planar DMA inputs).

Layout: PLANAR (feature-major). The host transposes/packs inputs into
  pcb  [7, NS]  = child(3) | prnt(3) | beta(1)
  eps  [9, NS]
and the device writes one packed output
  out8 [8, NS]  = c2c(3) | alpha(3) | copy_mask(1) | alpha_mask(1)
so each tile is 2 loads + 1 store and every operand is a contiguous
[128, T] plane (or a small strided stack of planes).

Device-side identities (validated vs reference to ~1e-5):
  - cos(p, cp) == cos(p, e) for e = exp(logits)  (scale invariance)
  - H(z/zs) = ln(zs) - (1/zs) * sum(z ln z), with z = relu(p+cp-.01)+.01
    and the +0.03 of zs folded into activation biases
  - 1/x (and rsqrt, and 42/x) as exp(-a*ln(x)+b) on the scalar engine
    (the custom-DVE fast reciprocal does not compile in this container)
  - norm guard: cos_raw = dot * rsqrt(sp*sc + 1e-30); dot==0 exactly when
    a norm is 0, matching the reference's where().

A post-pass (split_waits) hoists multi-semaphore waits onto dedicated
event-sem instructions: this container's walrus rejects instructions
carrying >1 sync wait.
"""

import os
import sys

sys.path.insert(0, "/opt/trn_rl_repo")

import numpy as np

import bass_rust
import concourse.bass as bass
import concourse.mybir as mybir
import concourse.tile as tile
from concourse.bass_utils import run_bass_kernel_spmd

PARTS = 128
T_COL = 784          # edges per partition per tile (even: bf16 2x align)
N_TILES = 5          # tiles per core
N_CORES = 8

f32 = mybir.dt.float32
bf16 = mybir.dt.bfloat16
Alu = mybir.AluOpType
Act = mybir.ActivationFunctionType

LAST_RESULT = None  # BassKernelResults of the most recent run (for test.py)

# setup_inputs() defaults, used if the harness omits the tiny tables.
_MU_DEFAULT = np.eye(3, dtype=np.float32)
_MU_DEFAULT[1, :] = [-0.25, 0.5, -0.25]

# ---------------------------------------------------------------------------
# split_waits post-pass
# ---------------------------------------------------------------------------
_WSPLIT_N = [0]


def _wait_carrier(engine, wait):
    _WSPLIT_N[0] += 1
    ev = mybir.InstEventSemaphore(name=f"WSPLIT-{_WSPLIT_N[0]}", ins=[], outs=[])
    ev.engine = engine
    ev.sync_info = bass_rust.SyncInfo(on_wait=[wait], on_update=[])
    return ev


def split_waits(nc, keep_on_control=1):
    for fn in nc.m.functions:
        for blk in fn.blocks:
            out = []
            for ins in blk.instructions:
                si = ins.sync_info
                waits = list(si.on_wait) if (si and si.on_wait) else []
                is_ctrl = type(ins).__name__ in (
                    "InstEventSemaphore",
                    "InstDrain",
                    "InstUnconditionalBranch",
                    "InstCompareAndBranch",
                    "InstBranchHint",
                )
                keep = keep_on_control if is_ctrl else 0
                if len(waits) > keep:
                    cut = len(waits) - keep
                    for w in waits[:cut]:
                        out.append(_wait_carrier(ins.engine, w))
                    ins.sync_info = bass_rust.SyncInfo(
                        on_wait=waits[cut:], on_update=list(si.on_update or [])
                    )
                out.append(ins)
            blk.instructions = out
    return nc



def build_graph(mu9, sg9, general, t_col=T_COL, n_tiles=N_TILES):
    """Per-core graph, planar layout.

    Fast path (degenerate tables): bf16 on every numerically-safe chain
    (logits/softmax/cosine/blend; DVE gets 2x-4x modes), f32 where rounding
    would amplify (ln values, entropy sums, z-sums). eps is cast f32->bf16
    by the SWDGE DMA in flight. GpSimd does no elementwise compute: it
    shares SBUF ports with the DVE and the two serialize, not overlap.
    """
    T = t_col
    NS = PARTS * T * n_tiles
    sigma_is_one = (not general) and bool(np.all(sg9 == 1.0))

    nc = bass.Bass()
    in_dt = f32 if general else bf16
    pcb_h = nc.declare_dram_parameter("pcb", [7, NS], in_dt, isOutput=False)
    eps_h = nc.declare_dram_parameter("eps", [9, NS], in_dt, isOutput=False)
    if general:
        murow_h = nc.declare_dram_parameter("murow", [9, NS], f32, isOutput=False)
        sgrow_h = nc.declare_dram_parameter("sgrow", [9, NS], f32, isOutput=False)
    outv_h = nc.declare_dram_parameter("outv", [6, NS], bf16, isOutput=True)
    outm_h = nc.declare_dram_parameter("outm", [2, NS], bf16, isOutput=True)

    def dram_tile(handle, base):
        return (
            handle[:, base : base + PARTS * T]
            .rearrange("k (p t) -> k p t", p=PARTS)
            .transpose([1, 0, 2])
        )

    with tile.TileContext(nc) as tc:
        with (
            tc.tile_pool(name="const", bufs=1) as cpool,
            tc.tile_pool(name="io", bufs=2) as io,
            tc.tile_pool(name="work", bufs=1) as wk,
            tc.tile_pool(name="work2", bufs=2) as wk2,
        ):
            bias_m01 = cpool.tile([PARTS, 1], f32)
            nc.gpsimd.memset(bias_m01[:], -0.01)
            bias_p01 = cpool.tile([PARTS, 1], f32)
            nc.gpsimd.memset(bias_p01[:], 0.01)
            bias_p03 = cpool.tile([PARTS, 1], f32)
            nc.gpsimd.memset(bias_p03[:], 0.03)
            bias_tiny = cpool.tile([PARTS, 1], f32)
            nc.gpsimd.memset(bias_tiny[:], 1e-30)
            bias_ln42 = cpool.tile([PARTS, 1], f32)
            nc.gpsimd.memset(bias_ln42[:], float(np.log(42.0)))

            t_plan = [T] * n_tiles
            base = 0
            T_MAX = T
            for it, t_cur in enumerate(t_plan):
                T = t_cur

                pdt = bf16 if not general else f32
                pcb_t = io.tile([PARTS, 7 * T], pdt, tag="pcb_t")
                nc.sync.dma_start(pcb_t[:], dram_tile(pcb_h, base))
                edt = bf16 if not general else f32
                e_t = io.tile([PARTS, 9 * T], edt, tag="e_t")
                nc.sync.dma_start(e_t[:], dram_tile(eps_h, base))
                if general:
                    mr_t = io.tile([PARTS, 9 * T], f32, tag="mr_t")
                    sr_t = io.tile([PARTS, 9 * T], f32, tag="sr_t")
                    nc.sync.dma_start(mr_t[:], dram_tile(murow_h, base))
                    nc.sync.dma_start(sr_t[:], dram_tile(sgrow_h, base))
                outv = io.tile([PARTS, 6 * T], bf16, tag="outv")
                outm = io.tile([PARTS, 2 * T], bf16, tag="outm")

                def sl(tl, k0, k1):
                    return tl[:, k0 * T : k1 * T]

                c3 = sl(pcb_t, 0, 3)
                p3 = sl(pcb_t, 3, 6)
                bt = sl(pcb_t, 6, 7)

                # fast path: pcb_t is already bf16; alias planes
                cpb6 = pcb_t[:, : 6 * T]
                cb3 = sl(pcb_t, 0, 3)
                pb3 = sl(pcb_t, 3, 6)
                bbf = sl(pcb_t, 6, 7)

                # ---- logits_i = sum_j (mu_ij + sg_ij*eps_ij)*c_j  [bf16] --
                epl = [sl(e_t, k, k + 1) for k in range(9)]
                if general:
                    nc.vector.tensor_tensor(e_t[:], e_t[:], sr_t[:], Alu.mult)
                    nc.vector.tensor_tensor(e_t[:], e_t[:], mr_t[:], Alu.add)
                elif not sigma_is_one:
                    for k in range(9):
                        nc.vector.tensor_scalar(
                            epl[k], epl[k], float(sg9[k]), float(mu9[k]),
                            Alu.mult, Alu.add,
                        )
                else:
                    for k in range(9):
                        # bf16 tensor_scalar: 4x mode
                        nc.vector.tensor_scalar(
                            epl[k], epl[k], float(mu9[k]), None, Alu.add
                        )
                e33 = e_t[:].rearrange("p (a b t) -> p a b t", a=3, b=3)
                for k in range(9):
                    nc.vector.tensor_tensor(
                        epl[k], epl[k], sl(pcb_t, k % 3, k % 3 + 1), Alu.mult
                    )
                # ---- exp / dots: exd = ex(3) | dpm(3)  [bf16] ------------
                exd = wk2.tile([PARTS, 6 * T], bf16, tag="exd")
                ex3t = sl(exd, 0, 3)
                lg3 = ex3t.rearrange("p (i t) -> p i t", i=3)
                nc.vector.tensor_tensor(
                    lg3, e33[:, :, 0, :], e33[:, :, 1, :], Alu.add
                )
                nc.vector.tensor_tensor(lg3, lg3, e33[:, :, 2, :], Alu.add)
                nc.scalar.activation(ex3t, ex3t, Act.Exp)
                nc.vector.tensor_tensor(sl(exd, 3, 6), pb3, ex3t, Alu.mult)
                sdp = wk.tile([PARTS, 2 * T], bf16, tag="sdp")
                sdp2 = sdp[:].rearrange("p (g t) -> p g t", g=2)
                exd23 = exd[:].rearrange("p (g i t) -> p g i t", g=2, i=3)
                nc.vector.tensor_tensor(
                    sdp2, exd23[:, :, 0, :], exd23[:, :, 1, :], Alu.add
                )
                nc.vector.tensor_tensor(sdp2, sdp2, exd23[:, :, 2, :], Alu.add)
                se = sl(sdp, 0, 1)
                dp = sl(sdp, 1, 2)

                # ---- r = 1/se (ln in f32!); cp = ex*r  [bf16] ------------
                rln = wk2.tile([PARTS, T], f32, tag="rln")
                nc.scalar.activation(rln[:], se, Act.Ln)
                r = wk2.tile([PARTS, T], bf16, tag="r")
                nc.scalar.activation(r[:], rln[:], Act.Exp, scale=-1.0)
                cp = wk.tile([PARTS, 3 * T], bf16, tag="cp")
                cp3 = cp[:].rearrange("p (i t) -> p i t", i=3)
                for i in range(3):
                    nc.vector.tensor_tensor(
                        sl(cp, i, i + 1), sl(exd, i, i + 1), r[:], Alu.mult
                    )

                # ---- masks (bf16 sums are exactly-zero iff f32 sums are) -
                cps2 = wk.tile([PARTS, 2 * T], bf16, tag="cps2")
                cps22 = cps2[:].rearrange("p (g t) -> p g t", g=2)
                cpb23 = cpb6.rearrange("p (g i t) -> p g i t", g=2, i=3)
                nc.vector.tensor_tensor(
                    cps22, cpb23[:, :, 0, :], cpb23[:, :, 1, :], Alu.add
                )
                nc.vector.tensor_tensor(
                    cps22, cps22, cpb23[:, :, 2, :], Alu.add
                )
                csum = sl(cps2, 0, 1)
                psum = sl(cps2, 1, 2)
                cm = wk.tile([PARTS, T], bf16, tag="cm")
                nc.vector.tensor_scalar(cm[:], csum, 0.0, None, Alu.not_equal)
                copym = sl(outm, 0, 1)
                amask = sl(outm, 1, 2)
                nc.vector.scalar_tensor_tensor(
                    copym, psum, 0.0, cm[:], Alu.is_equal, Alu.mult
                )
                nc.vector.tensor_tensor(amask, cm[:], copym, Alu.subtract)

                # ---- entropy: zz = z(3) | zlnz(3), all bf16 sums ---------
                zz = wk.tile([PARTS, 6 * T], bf16, tag="zz")
                zv = sl(zz, 0, 3)
                nc.vector.tensor_tensor(zv, pb3, cp[:], Alu.add)
                nc.scalar.activation(zv, zv, Act.Relu, bias=bias_m01[:])
                lnz = wk.tile([PARTS, 3 * T], f32, tag="lnz")
                nc.scalar.activation(lnz[:], zv, Act.Ln, bias=bias_p01[:])
                z01 = wk.tile([PARTS, 3 * T], f32, tag="z01")
                nc.scalar.activation(z01[:], zv, Act.Identity, bias=bias_p01[:])
                nc.vector.tensor_tensor(sl(zz, 3, 6), z01[:], lnz[:], Alu.mult)
                zst = wk.tile([PARTS, 2 * T], bf16, tag="zst")
                zst2 = zst[:].rearrange("p (g t) -> p g t", g=2)
                zz23 = zz[:].rearrange("p (g i t) -> p g i t", g=2, i=3)
                nc.vector.tensor_tensor(
                    zst2, zz23[:, :, 0, :], zz23[:, :, 1, :], Alu.add
                )
                nc.vector.tensor_tensor(zst2, zst2, zz23[:, :, 2, :], Alu.add)
                lnzs = wk.tile([PARTS, T], f32, tag="lnzs")
                nc.scalar.activation(lnzs[:], sl(zst, 0, 1), Act.Ln, bias=bias_p03[:])
                zr = wk.tile([PARTS, T], f32, tag="zr")
                nc.scalar.activation(zr[:], lnzs[:], Act.Exp, scale=-1.0)
                ent = wk.tile([PARTS, T], f32, tag="ent")
                nc.vector.tensor_tensor(ent[:], zr[:], sl(zst, 1, 2), Alu.mult)
                nc.vector.tensor_tensor(ent[:], lnzs[:], ent[:], Alu.subtract)
                eln = wk.tile([PARTS, T], f32, tag="eln")
                nc.scalar.activation(eln[:], ent[:], Act.Ln)
                esr = wk.tile([PARTS, T], bf16, tag="esr")  # 42/ent
                nc.scalar.activation(
                    esr[:], eln[:], Act.Exp, scale=-1.0, bias=bias_ln42[:]
                )

                # ---- cosine: sq = p^2(3)|ex^2(3)  [bf16] -----------------
                sq = wk.tile([PARTS, 6 * T], bf16, tag="sq")
                nc.scalar.activation(sl(sq, 0, 3), p3, Act.Square)
                nc.scalar.activation(sl(sq, 3, 6), ex3t, Act.Square)
                ssc = wk.tile([PARTS, 2 * T], bf16, tag="ssc")
                ssc2 = ssc[:].rearrange("p (g t) -> p g t", g=2)
                sq23 = sq[:].rearrange("p (g i t) -> p g i t", g=2, i=3)
                nc.vector.tensor_tensor(
                    ssc2, sq23[:, :, 0, :], sq23[:, :, 1, :], Alu.add
                )
                nc.vector.tensor_tensor(ssc2, ssc2, sq23[:, :, 2, :], Alu.add)
                mn = wk.tile([PARTS, T], bf16, tag="mn")
                nc.vector.tensor_tensor(
                    mn[:], sl(ssc, 0, 1), sl(ssc, 1, 2), Alu.mult
                )
                mnl = wk.tile([PARTS, T], f32, tag="mnl")
                nc.scalar.activation(mnl[:], mn[:], Act.Ln, bias=bias_tiny[:])
                drr = wk.tile([PARTS, T], bf16, tag="drr")
                nc.scalar.activation(drr[:], mnl[:], Act.Exp, scale=-0.5)

                # ---- sm = (max(.01, dp*drr) * (42/ent)) * amask [bf16] ---
                sm = wk.tile([PARTS, T], bf16, tag="sm")
                nc.vector.tensor_tensor(sm[:], dp, drr[:], Alu.mult)
                nc.vector.scalar_tensor_tensor(
                    sm[:], sm[:], 0.01, esr[:], Alu.max, Alu.mult
                )
                nc.vector.tensor_tensor(sm[:], sm[:], amask, Alu.mult)

                # ---- alpha = (p + beta*(cp-p))*sm ; c2c = cp*copym -------
                av = sl(outv, 3, 6)
                nc.vector.tensor_tensor(av, cp[:], pb3, Alu.subtract)
                for i in range(3):
                    avi = sl(outv, 3 + i, 4 + i)
                    nc.vector.tensor_tensor(avi, avi, bbf, Alu.mult)
                nc.vector.tensor_tensor(av, av, pb3, Alu.add)
                for i in range(3):
                    avi = sl(outv, 3 + i, 4 + i)
                    nc.vector.tensor_tensor(avi, avi, sm[:], Alu.mult)
                for i in range(3):
                    nc.vector.tensor_tensor(
                        sl(outv, i, i + 1), sl(cp, i, i + 1), copym, Alu.mult
                    )

                nc.sync.dma_start(dram_tile(outv_h, base), outv[:])
                nc.sync.dma_start(dram_tile(outm_h, base), outm[:])
                base += PARTS * T

    split_waits(nc)
    return nc, NS


def _pack_planar(arrs, ns, dtype=np.float32):
    """[N, F_a] arrays -> list of 8 contiguous [sum F, ns] shards (padded)."""
    n = arrs[0].shape[0]
    ftot = sum(a.shape[1] for a in arrs)
    shards = []
    for i in range(N_CORES):
        lo, hi = i * ns, (i + 1) * ns
        out = np.zeros((ftot, ns), dtype=dtype)
        m = max(0, min(hi, n) - lo)
        if m > 0:
            k = 0
            for a in arrs:
                f = a.shape[1]
                out[k : k + f, :m] = a[lo : lo + m].T
                k += f
        shards.append(out)
    return shards


def kernel(
    prnt_probs,
    child_probs,
    eps_M,
    beta,
    rels=None,
    rel_mu=None,
    rel_sigma=None,
    **_unused,
):
    global LAST_RESULT
    prnt = np.asarray(prnt_probs, dtype=np.float32)
    child = np.asarray(child_probs, dtype=np.float32)
    n = prnt.shape[0]
    eps = np.asarray(eps_M, dtype=np.float32).reshape(n, 9)
    bet = np.asarray(beta, dtype=np.float32).reshape(n, 1)

    if rel_mu is None:
        rel_mu = np.tile(_MU_DEFAULT, (20, 1)).reshape(20, 3, 3)
    if rel_sigma is None:
        rel_sigma = np.ones((20, 3, 3), dtype=np.float32)
    rel_mu = np.asarray(rel_mu, dtype=np.float32)
    rel_sigma = np.asarray(rel_sigma, dtype=np.float32)

    degenerate = bool(
        np.all(rel_mu == rel_mu[0:1]) and np.all(rel_sigma == rel_sigma[0:1])
    )

    t_col = T_COL if degenerate else 360  # general path: f32 tiles, smaller T
    n_tiles = max(1, -(-n // (N_CORES * PARTS * t_col)))  # ceil
    nc, ns = build_graph(
        rel_mu[0].reshape(9),
        rel_sigma[0].reshape(9),
        general=not degenerate,
        t_col=t_col,
        n_tiles=n_tiles,
    )

    import ml_dtypes

    in_np = np.float32 if not degenerate else ml_dtypes.bfloat16
    pcb_sh = _pack_planar([child, prnt, bet], ns, dtype=in_np)
    eps_sh = _pack_planar([eps], ns, dtype=in_np)
    if not degenerate:
        ridx = np.asarray(rels).astype(np.int64)
        mr_sh = _pack_planar([rel_mu.reshape(20, 9)[ridx]], ns)
        sr_sh = _pack_planar([rel_sigma.reshape(20, 9)[ridx]], ns)
    in_maps = []
    for i in range(N_CORES):
        m = {"pcb": pcb_sh[i], "eps": eps_sh[i]}
        if not degenerate:
            m["murow"] = mr_sh[i]
            m["sgrow"] = sr_sh[i]
        in_maps.append(m)

    trace = bool(os.environ.get("ALPHA_KERNEL_TRACE"))
    res = run_bass_kernel_spmd(
        nc, in_maps, core_ids=list(range(N_CORES)), trace=trace
    )
    LAST_RESULT = res
    outs = res.results

    ov = np.concatenate([outs[i]["outv"] for i in range(N_CORES)], axis=1)
    om = np.concatenate([outs[i]["outm"] for i in range(N_CORES)], axis=1)
    c2c = np.ascontiguousarray(ov[0:3, :n].T.astype(np.float32))
    alpha = np.ascontiguousarray(ov[3:6, :n].T.astype(np.float32))
    copy_mask = om[0, :n] != 0
    alpha_mask = om[1, :n] != 0
    return copy_mask, c2c, alpha_mask, alpha
